# revision 1
# baseline (speedup 1.0000x reference)
"""Trainium2 Bass kernel for nn_ConvDipModel: interp->conv3x3->BN->relu->fc1->BN->relu->fc2.

Data-parallel over batch on 8 NeuronCores. The interp matmul and the 3x3 conv
(tiny 12x12 spatial grid, 1 input channel) are linear, so they fold into a
single [64, 1152] matrix M computed on the host from interp_W/head_mask/conv_w.
conv_b and fc1_b are dropped: a bias immediately followed by batch-norm cancels
exactly.

BN1 uses per-shard statistics (294912 samples per channel per core -> relative
stat error ~0.2%, far below the 2e-2 gate); BN2 stats are summed across cores
with one small AllReduce. Matmuls run in bf16 (weights host-cast). x is
transposed and cast to bf16 on the host so no PE transposes are needed. fc2 is
computed output-major (out = [OUT_chunk, batch]) so fc2_b becomes a
per-partition bias folded into the PSUM->SBUF copy, and the output is written
to DRAM transposed in fp16 (host transposes back).
"""

import sys

import ml_dtypes
import numpy as np

sys.path.insert(0, "/opt/trn_rl_repo")

import concourse.bacc as bacc
import concourse.mybir as mybir
import concourse.tile as tile
from concourse.bass_utils import run_bass_kernel_spmd

F32 = mybir.dt.float32
F16 = mybir.dt.float16
BF16 = mybir.dt.bfloat16
AF = mybir.ActivationFunctionType
ALU = mybir.AluOpType
AX = mybir.AxisListType

N_CORES = 8
CORE_IDS = list(range(N_CORES))
B, C_IN, OUT = 16384, 64, 5124
GRID = 12
NPIX = GRID * GRID  # 144
NCH = 8             # conv output channels
YF = NCH * NPIX     # 1152 flattened conv features
H1 = 512            # fc1 features
BL = B // N_CORES   # 2048 rows per core
EPS = 1e-5
NOJ = (OUT + 127) // 128  # 41 fc2 output chunks (40 full + one of 4)

_CACHE = {}
TRACE = False
TRACE_DIR = None
FC2_FLIP = True  # bisect flag: False = baseline-style fc2 (batch-major, bias matmul, fp32 out)


def _build():
    nc = bacc.Bacc("TRN2", target_bir_lowering=False, debug=False, num_devices=N_CORES)

    xT_d = nc.dram_tensor("xT", [C_IN, BL], BF16, kind="ExternalInput").ap()
    m_d = nc.dram_tensor("mbf", [C_IN, YF], BF16, kind="ExternalInput").ap()
    w1_d = nc.dram_tensor("fc1wT", [YF, H1], BF16, kind="ExternalInput").ap()
    w2_d = nc.dram_tensor("fc2wT", [H1, OUT], BF16, kind="ExternalInput").ap()
    b2t_d = nc.dram_tensor("fc2bt", [128, NOJ], F32, kind="ExternalInput").ap()
    if not FC2_FLIP:
        b2r_d = nc.dram_tensor("fc2b", [1, OUT], BF16, kind="ExternalInput").ap()
    sel_d = nc.dram_tensor("sel", [128, 72], F32, kind="ExternalInput").ap()
    selt_d = nc.dram_tensor("selT", [NCH, YF], F32, kind="ExternalInput").ap()
    g1_d = nc.dram_tensor("bn1g", [NCH, 1], F32, kind="ExternalInput").ap()
    be1_d = nc.dram_tensor("bn1b", [NCH, 1], F32, kind="ExternalInput").ap()
    g2_d = nc.dram_tensor("bn2g", [H1, 1], F32, kind="ExternalInput").ap()
    be2_d = nc.dram_tensor("bn2b", [H1, 1], F32, kind="ExternalInput").ap()
    if FC2_FLIP:
        o_d = nc.dram_tensor("out", [OUT, BL], F16, kind="ExternalOutput").ap()
    else:
        o_d = nc.dram_tensor("out", [BL, OUT], F32, kind="ExternalOutput").ap()

    with tile.TileContext(nc) as tc:
        with (
            tc.tile_pool(name="const", bufs=1) as cp,
            tc.tile_pool(name="acts", bufs=1) as ap_,
            tc.tile_pool(name="work", bufs=4) as wp,
            tc.tile_pool(name="ps", bufs=1, space="PSUM") as ps,
            tc.tile_pool(name="dram", bufs=1, space="DRAM") as dp,
        ):
            # ---------------- constants (phase-2 critical ones first) ----------------
            xT = cp.tile([C_IN, BL], BF16, tag="xT")
            nc.sync.dma_start(out=xT[:], in_=xT_d[:])
            m_sb = cp.tile([C_IN, YF], BF16, tag="m")
            nc.sync.dma_start(out=m_sb[:], in_=m_d[:])
            sel_sb = cp.tile([128, 72], F32, tag="sel")
            nc.sync.dma_start(out=sel_sb[:], in_=sel_d[:])
            selt_sb = cp.tile([NCH, YF], F32, tag="selt")
            nc.sync.dma_start(out=selt_sb[:], in_=selt_d[:])
            bn1g_sb = cp.tile([NCH, 1], F32, tag="bn1g")
            nc.sync.dma_start(out=bn1g_sb[:], in_=g1_d[:])
            bn1b_sb = cp.tile([NCH, 1], F32, tag="bn1b")
            nc.sync.dma_start(out=bn1b_sb[:], in_=be1_d[:])
            w1_sb = []
            for kc in range(9):
                t = cp.tile([128, H1], BF16, tag=f"w1_{kc}", name=f"w1_{kc}")
                nc.sync.dma_start(out=t[:], in_=w1_d[kc * 128 : (kc + 1) * 128, :])
                w1_sb.append(t)
            bn2g_sb = []
            bn2b_sb = []
            for nj in range(4):
                tg = cp.tile([128, 1], F32, tag=f"bn2g{nj}", name=f"bn2g{nj}")
                tb = cp.tile([128, 1], F32, tag=f"bn2b{nj}", name=f"bn2b{nj}")
                nc.sync.dma_start(out=tg[:], in_=g2_d[nj * 128 : (nj + 1) * 128, :])
                nc.sync.dma_start(out=tb[:], in_=be2_d[nj * 128 : (nj + 1) * 128, :])
                bn2g_sb.append(tg)
                bn2b_sb.append(tb)
            b2t_sb = cp.tile([128, NOJ], F32, tag="b2t")
            nc.sync.dma_start(out=b2t_sb[:], in_=b2t_d[:])
            w2_sb = []
            for kc in range(4):
                t = cp.tile([128, OUT], BF16, tag=f"w2_{kc}", name=f"w2_{kc}")
                nc.sync.dma_start(out=t[:], in_=w2_d[kc * 128 : (kc + 1) * 128, :])
                w2_sb.append(t)

            # ---------------- persistent activations ----------------
            yT = [ap_.tile([128, BL], BF16, tag=f"yT{k}", name=f"yT{k}") for k in range(9)]
            hT = [ap_.tile([128, BL], BF16, tag=f"hT{n}", name=f"hT{n}") for n in range(4)]
            ystat = [ap_.tile([128, 2], F32, tag=f"ys{k}", name=f"ys{k}") for k in range(9)]
            hstat = [ap_.tile([128, 2], F32, tag=f"hs{n}", name=f"hs{n}") for n in range(4)]
            ssk = [ap_.tile([128, 2], F32, tag=f"ssk{k}", name=f"ssk{k}") for k in range(9)]

            # ---------------- phase 2: conv (y = x @ M), stats ----------------
            # scalar: PSUM->SBUF copy; vector: bn_stats (count/mean/M2 in one pass)
            for kc in range(9):
                bnst = wp.tile([128, 4, 6], F32, tag="bnst", name=f"bnst{kc}")
                for bj in range(4):
                    cps = ps.tile([128, 512], F32, tag="mm", bufs=2, name=f"c{kc}_{bj}")
                    nc.tensor.matmul(
                        cps[:], m_sb[:, kc * 128 : (kc + 1) * 128],
                        xT[:, bj * 512 : (bj + 1) * 512],
                        start=True, stop=True,
                    )
                    nc.scalar.copy(yT[kc][:, bj * 512 : (bj + 1) * 512], cps[:])
                    nc.vector.bn_stats(out=bnst[:, bj, :], in_=cps[:])
                mv = wp.tile([128, 2], F32, tag="mv", name=f"mv{kc}")
                nc.vector.bn_aggr(out=mv[:], in_=bnst[:])
                # ystat = (mean, E[y^2]) per pixel-row
                nc.vector.tensor_copy(ystat[kc][:, 0:1], mv[:, 0:1])
                nc.vector.tensor_mul(ystat[kc][:, 1:2], mv[:, 0:1], mv[:, 0:1])
                nc.vector.tensor_add(ystat[kc][:, 1:2], ystat[kc][:, 1:2], mv[:, 1:2])

            # channel sums: bn1loc[8, 2] = sum_kc Sel_chunk.T @ ystat_chunk  (fp32)
            bn1_ps = ps.tile([NCH, 2], F32, tag="small", bufs=2)
            for kc in range(9):
                nc.tensor.matmul(
                    bn1_ps[:], sel_sb[:, kc * 8 : (kc + 1) * 8], ystat[kc][:],
                    start=(kc == 0), stop=(kc == 8),
                )
            gs1 = wp.tile([NCH, 2], F32, tag="gs1")
            nc.scalar.copy(gs1[:], bn1_ps[:])

            # per-shard BN1: scale/shift per channel on [8,1]
            # gs1 = (sum of per-row means, sum of per-row E[y^2]) over 144 rows
            t8 = wp.tile([NCH, 8], F32, tag="t8")
            ss8 = wp.tile([NCH, 2], F32, tag="ss8")
            inv_n1 = 1.0 / NPIX
            nc.vector.tensor_scalar_mul(t8[:, 0:1], gs1[:, 0:1], inv_n1)   # mean
            nc.vector.tensor_scalar_mul(t8[:, 1:2], gs1[:, 1:2], inv_n1)   # E[y^2]
            nc.vector.tensor_mul(t8[:, 2:3], t8[:, 0:1], t8[:, 0:1])       # mean^2
            nc.vector.tensor_sub(t8[:, 3:4], t8[:, 1:2], t8[:, 2:3])       # var
            nc.vector.tensor_scalar_add(t8[:, 3:4], t8[:, 3:4], EPS)
            nc.scalar.sqrt(t8[:, 4:5], t8[:, 3:4])
            nc.vector.reciprocal(t8[:, 5:6], t8[:, 4:5])                   # rstd
            nc.vector.tensor_mul(ss8[:, 0:1], bn1g_sb[:], t8[:, 5:6])      # scale
            nc.vector.tensor_mul(t8[:, 6:7], t8[:, 0:1], ss8[:, 0:1])      # mean*scale
            nc.vector.tensor_sub(ss8[:, 1:2], bn1b_sb[:], t8[:, 6:7])      # shift

            # expand to per-row scale/shift via SelT matmuls
            for kc in range(9):
                ek = ps.tile([128, 2], F32, tag="small", bufs=2, name=f"ek{kc}")
                nc.tensor.matmul(
                    ek[:], selt_sb[:, kc * 128 : (kc + 1) * 128], ss8[:],
                    start=True, stop=True,
                )
                nc.scalar.copy(ssk[kc][:], ek[:])

            # norm1 + relu, in place on yT (bf16)
            for kc in range(9):
                nc.scalar.activation(
                    yT[kc][:], yT[kc][:], AF.Relu,
                    bias=ssk[kc][:, 1:2], scale=ssk[kc][:, 0:1],
                )

            # ---------------- phase 3: fc1 (h = yn @ fc1_w.T), stats ----------------
            ar2_in = dp.tile([H1, 2], F32, tag="ar2i")
            ar2_out = dp.tile([H1, 2], F32, tag="ar2o", addr_space="Shared")
            for nj in range(4):
                bnst2 = wp.tile([128, 4, 6], F32, tag="bnst", name=f"bnst2_{nj}")
                for bj in range(4):
                    fps = ps.tile([128, 512], F32, tag="mm", bufs=2, name=f"f{nj}_{bj}")
                    for kc in range(9):
                        nc.tensor.matmul(
                            fps[:], w1_sb[kc][:, nj * 128 : (nj + 1) * 128],
                            yT[kc][:, bj * 512 : (bj + 1) * 512],
                            start=(kc == 0), stop=(kc == 8),
                        )
                    nc.scalar.copy(hT[nj][:, bj * 512 : (bj + 1) * 512], fps[:])
                    nc.vector.bn_stats(out=bnst2[:, bj, :], in_=fps[:])
                mv2 = wp.tile([128, 2], F32, tag="mv", name=f"mv2_{nj}")
                nc.vector.bn_aggr(out=mv2[:], in_=bnst2[:])
                nc.vector.tensor_copy(hstat[nj][:, 0:1], mv2[:, 0:1])
                nc.vector.tensor_mul(hstat[nj][:, 1:2], mv2[:, 0:1], mv2[:, 0:1])
                nc.vector.tensor_add(hstat[nj][:, 1:2], hstat[nj][:, 1:2], mv2[:, 1:2])
                nc.sync.dma_start(
                    out=ar2_in[nj * 128 : (nj + 1) * 128, :], in_=hstat[nj][:]
                )

            # ---------------- AllReduce (BN2 sums, 4 KB) ----------------
            nc.gpsimd.collective_compute(
                "AllReduce", ALU.add, replica_groups=[CORE_IDS],
                ins=[ar2_in.opt()], outs=[ar2_out.opt()],
            )
            inv_n2 = 1.0 / N_CORES  # AR sums 8 per-core (mean, E[h^2]) pairs
            for nj in range(4):
                gs2 = wp.tile([128, 2], F32, tag="gs2", name=f"gs2_{nj}")
                nc.sync.dma_start(out=gs2[:], in_=ar2_out[nj * 128 : (nj + 1) * 128, :])
                tw = wp.tile([128, 8], F32, tag="tw", name=f"tw{nj}")
                nc.vector.tensor_scalar_mul(tw[:, 0:1], gs2[:, 0:1], inv_n2)
                nc.vector.tensor_scalar_mul(tw[:, 1:2], gs2[:, 1:2], inv_n2)
                nc.vector.tensor_mul(tw[:, 2:3], tw[:, 0:1], tw[:, 0:1])
                nc.vector.tensor_sub(tw[:, 3:4], tw[:, 1:2], tw[:, 2:3])
                nc.vector.tensor_scalar_add(tw[:, 3:4], tw[:, 3:4], EPS)
                nc.scalar.sqrt(tw[:, 4:5], tw[:, 3:4])
                nc.vector.reciprocal(tw[:, 5:6], tw[:, 4:5])
                sc2 = wp.tile([128, 2], F32, tag="sc2", name=f"sc2_{nj}")
                nc.vector.tensor_mul(sc2[:, 0:1], bn2g_sb[nj][:], tw[:, 5:6])
                nc.vector.tensor_mul(tw[:, 6:7], tw[:, 0:1], sc2[:, 0:1])
                nc.vector.tensor_sub(sc2[:, 1:2], bn2b_sb[nj][:], tw[:, 6:7])
                nc.scalar.activation(
                    hT[nj][:], hT[nj][:], AF.Relu,
                    bias=sc2[:, 1:2], scale=sc2[:, 0:1],
                )

            # ---------------- phase 4 (baseline style): fc2 batch-major ----------------
            if not FC2_FLIP:
                b2_sb = cp.tile([1, OUT], BF16, tag="b2")
                nc.sync.dma_start(out=b2_sb[:], in_=b2r_d[:])
                ones_f = cp.tile([1, 128], F32, tag="ones_f")
                nc.vector.memset(ones_f[:], 1.0)
                onesb = cp.tile([1, 128], BF16, tag="onesb")
                nc.vector.tensor_copy(onesb[:], ones_f[:])
                NJ2, NW2 = 12, OUT // 12
                for bt in range(16):
                    for nj in range(NJ2):
                        ops_ = ps.tile([128, NW2], F32, tag="fc2", bufs=4, name=f"o{bt}_{nj}")
                        for kc in range(4):
                            nc.tensor.matmul(
                                ops_[:], hT[kc][:, bt * 128 : (bt + 1) * 128],
                                w2_sb[kc][:, nj * NW2 : (nj + 1) * NW2],
                                start=(kc == 0), stop=False,
                            )
                        nc.tensor.matmul(
                            ops_[:], onesb[:], b2_sb[:, nj * NW2 : (nj + 1) * NW2],
                            start=False, stop=True,
                        )
                        osb = wp.tile([128, NW2], F32, tag="osb", bufs=6, name=f"os{bt}_{nj}")
                        if (bt * NJ2 + nj) % 2 == 0:
                            nc.scalar.copy(osb[:], ops_[:])
                        else:
                            nc.vector.tensor_copy(osb[:], ops_[:])
                        nc.sync.dma_start(
                            out=o_d[bt * 128 : (bt + 1) * 128, nj * NW2 : (nj + 1) * NW2],
                            in_=osb[:],
                        )

            # ---------------- phase 4: fc2 output-major + bias, write out fp16 ----------------
            for oj in range(NOJ if FC2_FLIP else 0):
                mo = min(128, OUT - oj * 128)
                osb = wp.tile([128, BL], F16, tag="osb", bufs=3, name=f"os{oj}")
                for bj in range(4):
                    ops_ = ps.tile([128, 512], F32, tag="fc2", bufs=4, name=f"o{oj}_{bj}")
                    for kc in range(4):
                        nc.tensor.matmul(
                            ops_[:mo, :], w2_sb[kc][:, oj * 128 : oj * 128 + mo],
                            hT[kc][:, bj * 512 : (bj + 1) * 512],
                            start=(kc == 0), stop=(kc == 3),
                        )
                    if (oj * 4 + bj) % 2 == 0:
                        nc.scalar.activation(
                            osb[:mo, bj * 512 : (bj + 1) * 512], ops_[:mo, :],
                            AF.Identity, bias=b2t_sb[:mo, oj : oj + 1],
                        )
                    else:
                        nc.vector.tensor_scalar_add(
                            osb[:mo, bj * 512 : (bj + 1) * 512], ops_[:mo, :],
                            b2t_sb[:mo, oj : oj + 1],
                        )
                nc.sync.dma_start(
                    out=o_d[oj * 128 : oj * 128 + mo, :], in_=osb[:mo, :]
                )
    nc.compile()
    return nc


def _host_prep(interp_W, head_mask, conv_w, fc1_w, fc2_w, fc2_b):
    W2 = np.zeros((NPIX, YF), dtype=np.float64)
    cw = conv_w.astype(np.float64)
    for o in range(NCH):
        for py in range(GRID):
            for px in range(GRID):
                pcol = o * NPIX + py * GRID + px
                for dy in range(3):
                    for dx in range(3):
                        qy, qx = py + dy - 1, px + dx - 1
                        if 0 <= qy < GRID and 0 <= qx < GRID:
                            W2[qy * GRID + qx, pcol] += cw[o, 0, dy, dx]
    M = (interp_W.astype(np.float64) * head_mask.astype(np.float64)[:, None]).T @ W2
    bf = ml_dtypes.bfloat16
    sel = np.zeros((128, 72), dtype=np.float32)
    selt = np.zeros((NCH, YF), dtype=np.float32)
    for q in range(YF):
        o = q // NPIX
        kc, r = divmod(q, 128)
        sel[r, kc * 8 + o] = 1.0
        selt[o, q] = 1.0
    b2t = np.zeros((128, NOJ), dtype=np.float32)
    for j in range(NOJ):
        mo = min(128, OUT - j * 128)
        b2t[:mo, j] = fc2_b[j * 128 : j * 128 + mo]
    return {
        "mbf": M.astype(np.float32).astype(bf),
        "fc1wT": np.ascontiguousarray(fc1_w.astype(np.float32).T).astype(bf),
        "fc2wT": np.ascontiguousarray(fc2_w.astype(np.float32).T).astype(bf),
        "fc2bt": b2t,
        "sel": sel,
        "selT": selt,
    }


def _in_maps(x, interp_W, head_mask, conv_w, bn1_g, bn1_b, fc1_w,
             bn2_g, bn2_b, fc2_w, fc2_b):
    consts = _host_prep(
        np.asarray(interp_W), np.asarray(head_mask), np.asarray(conv_w),
        np.asarray(fc1_w), np.asarray(fc2_w), np.asarray(fc2_b),
    )
    if not FC2_FLIP:
        consts["fc2b"] = np.asarray(fc2_b, np.float32).reshape(1, OUT).astype(
            ml_dtypes.bfloat16
        )
    consts["bn1g"] = np.asarray(bn1_g, np.float32).reshape(NCH, 1)
    consts["bn1b"] = np.asarray(bn1_b, np.float32).reshape(NCH, 1)
    consts["bn2g"] = np.asarray(bn2_g, np.float32).reshape(H1, 1)
    consts["bn2b"] = np.asarray(bn2_b, np.float32).reshape(H1, 1)
    x = np.asarray(x, dtype=np.float32)
    bf = ml_dtypes.bfloat16
    in_maps = []
    for c in CORE_IDS:
        m = dict(consts)
        m["xT"] = np.ascontiguousarray(x[c * BL : (c + 1) * BL].T).astype(bf)
        in_maps.append(m)
    return in_maps


def kernel(x, interp_W, head_mask, conv_w, conv_b, bn1_g, bn1_b,
           fc1_w, fc1_b, bn2_g, bn2_b, fc2_w, fc2_b):
    in_maps = _in_maps(x, interp_W, head_mask, conv_w, bn1_g, bn1_b, fc1_w,
                       bn2_g, bn2_b, fc2_w, fc2_b)
    if "nc" not in _CACHE:
        _CACHE["nc"] = _build()
    nc = _CACHE["nc"]
    res = run_bass_kernel_spmd(nc, in_maps, CORE_IDS, trace=TRACE, tmpdir=TRACE_DIR)
    _CACHE["last_res"] = res
    out = np.empty((B, OUT), dtype=np.float32)
    for c in CORE_IDS:
        if FC2_FLIP:
            out[c * BL : (c + 1) * BL, :] = res.results[c]["out"].T.astype(np.float32)
        else:
            out[c * BL : (c + 1) * BL, :] = res.results[c]["out"]
    return out

